# revision 50
# baseline (speedup 1.0000x reference)
"""MLA (DeepSeek-style multi-head latent attention) forward on 8 trn2 cores.

Layout v2: sequence-sharded LoRA-A + device collectives + bf16 matmuls.

Each core computes the LoRA-A projections (q_latent, compressed-kv latent,
k_pe) only for its 256-column sequence shard (8x less replicated work than
pure head-TP). The normalized kv latent + rope'd k_pe are AllGathered
(shared by every head); the per-head q vectors are redistributed with two
AllToAlls (one per head of each core's head pair) so attention runs fully
head-local: core c owns heads 2c, 2c+1 over the full sequence. kv_b expands
kn/v from the gathered latent per head; o_proj is input-split on heads and
the partial products are summed on the host (the unshard step).

All matmuls run in bf16 (1 PE cycle/row regardless of free-dim size, half
the DMA/communication bytes of fp32; final accuracy ~4e-3 vs the 2e-2
gate). Softmax runs over the key (partition) axis: exp on the scalar
engine, denominator via a ones-column matmul, broadcast of per-column
scalars via a K=1 matmul. RoPE rotate-half is a matmul against a constant
signed permutation. o_proj results are DMA'd directly from PSUM.
"""
import numpy as np
import ml_dtypes

import concourse.bass as bass
import concourse.tile as tile
from concourse import bacc, mybir
from concourse.bass_utils import run_bass_kernel_spmd

F32 = mybir.dt.float32
BF16 = mybir.dt.bfloat16
NPBF = ml_dtypes.bfloat16

HID = 2048
S = 2048
H = 16
QL = 1536
KVL = 512
NOPE = 128
RP = 64
VD = 128
QD = NOPE + RP              # 192
SCALE = QD ** -0.5
EPS = 1e-6
ROPE_THETA = 10000.0

NC = 8
HPC = 2                     # heads per core
SSH = S // NC               # 256-seq shard
KT = HID // 128             # 16
QLT = QL // 128             # 12
CT = KVL // 128             # 4
SB = 512                    # attention query block
NSB = S // SB               # 4
NEG = -30000.0

_CACHE = {}
LAST_RESULT = None


def _build_program():
    nc = bacc.Bacc("TRN2", target_bir_lowering=False, debug=False,
                   num_devices=NC)
    d_xt = nc.dram_tensor("xt16", [128, KT, SSH], BF16, kind="ExternalInput").ap()
    d_wqa = nc.dram_tensor("wqa16", [128, KT, QL], BF16, kind="ExternalInput").ap()
    d_wkva = nc.dram_tensor("wkva16", [128, KT, KVL + RP], BF16, kind="ExternalInput").ap()
    d_wqb = nc.dram_tensor("wqb16", [128, QLT, H * QD], BF16, kind="ExternalInput").ap()
    d_wk = nc.dram_tensor("wk16", [128, CT, HPC * NOPE], BF16, kind="ExternalInput").ap()
    d_wv = nc.dram_tensor("wv16", [128, CT, HPC * VD], BF16, kind="ExternalInput").ap()
    d_wo = nc.dram_tensor("wo16", [128, HPC, HID], BF16, kind="ExternalInput").ap()
    d_cos = nc.dram_tensor("cosd", [128, SSH], BF16, kind="ExternalInput").ap()
    d_sin = nc.dram_tensor("sind", [128, SSH], BF16, kind="ExternalInput").ap()
    d_msk = nc.dram_tensor("maskadd", [128, 4, SB], F32, kind="ExternalInput").ap()
    d_rotq = nc.dram_tensor("rotq16", [128, 128], BF16, kind="ExternalInput").ap()
    d_out = nc.dram_tensor("out", [S, HID], BF16, kind="ExternalOutput").ap()

    with tile.TileContext(nc) as tc:
        _mla(tc, d_xt, d_wqa, d_wkva, d_wqb, d_wk, d_wv, d_wo, d_cos, d_sin,
             d_msk, d_rotq, d_out)
    nc.compile()
    return nc


def _mla(tc, d_xt, d_wqa, d_wkva, d_wqb, d_wk, d_wv, d_wo, d_cos, d_sin,
         d_msk, d_rotq, d_out):
    nc = tc.nc
    Exp = mybir.ActivationFunctionType.Exp
    Sqrt = mybir.ActivationFunctionType.Sqrt
    groups = [list(range(NC))]

    with nc.allow_low_precision(reason="bf16 pipeline"), \
         tc.tile_pool(name="pdram", bufs=1, space="DRAM") as pdram, \
         tc.tile_pool(name="pconst", bufs=1) as pc, \
         tc.tile_pool(name="pglob", bufs=1) as pg:
        # ---- DRAM bounce buffers for collectives ----
        ag_in = pdram.tile([KVL + RP, SSH], BF16)
        ag_out = pdram.tile([NC, KVL + RP, SSH], BF16)
        aa_in = [pdram.tile([NC, QD, SSH], BF16, name=f"aain{i}") for i in range(HPC)]
        aa_out = [pdram.tile([NC, QD, SSH], BF16, name=f"aaout{i}") for i in range(HPC)]

        # ---- small constants ----
        ones_c = pc.tile([128, 1], BF16)
        nc.vector.memset(ones_c, 1.0)
        ones_r = pc.tile([1, 128], BF16)
        nc.vector.memset(ones_r, 1.0)
        eps1 = pc.tile([1, 1], F32)
        nc.vector.memset(eps1, EPS)
        warm = pc.tile([1, 2], F32, name="actwarm")
        nc.scalar.activation(warm[0:1, 0:1], eps1[:], Sqrt)
        nc.scalar.activation(warm[0:1, 1:2], eps1[:], Exp)
        rotq = pc.tile([128, 128], BF16)
        nc.sync.dma_start(out=rotq, in_=d_rotq)
        cosd = pc.tile([128, SSH], BF16)
        nc.sync.dma_start(out=cosd, in_=d_cos)
        sind = pc.tile([128, SSH], BF16)
        nc.sync.dma_start(out=sind, in_=d_sin)

        # =============== stage A: shard projections ===============
        with tc.tile_pool(name="pw", bufs=1) as pw, \
             tc.tile_pool(name="pA", bufs=1) as pa, \
             tc.tile_pool(name="pAq", bufs=3) as paq, \
             tc.tile_pool(name="pAs", bufs=3) as pas, \
             tc.tile_pool(name="ppA", bufs=3, space="PSUM") as ppa, \
             tc.tile_pool(name="ppSt", bufs=2, space="PSUM") as ppst, \
             tc.tile_pool(name="ppM", bufs=2, space="PSUM") as ppm:
            # ---- stage-A weights: per-k-slice tiles so matmuls start early ----
            xt = pw.tile([128, KT, SSH], BF16)
            # Early (eager) loads on the SP HW queue: only what the first
            # ~20us of compute needs. Everything else is loaded via gpsimd
            # SWDGE triggers placed AFTER the AllGather in program order, so
            # those transfers enter the global DMA FIFO behind the
            # collective staging instead of ahead of it.
            wkva = pw.tile([128, KT, KVL + RP], BF16)
            for half in range(2):
                hk = slice(half * KT // 2, (half + 1) * KT // 2)
                nc.sync.dma_start(out=xt[:, hk, :], in_=d_xt[:, hk, :])
                nc.sync.dma_start(out=wkva[:, hk, :], in_=d_wkva[:, hk, :])
            wqa = pw.tile([128, KT, QL], BF16)
            # gate wqa-colA behind wkva via a write-after-read dep: the
            # reader consumes both the colA region and the wkva tail, so the
            # colA DMA (a writer of that region) must wait for wkva.
            gate = pc.tile([1, 2], BF16, name="gate")
            nc.vector.tensor_tensor(gate[0:1, 0:1], wqa[0:1, 0, 0:1],
                                    wkva[0:1, KT - 1, 0:1], mybir.AluOpType.mult)
            nc.scalar.dma_start(out=wqa[:, :, 0:QL // 2], in_=d_wqa[:, :, 0:QL // 2])
            wqb = pw.tile([128, QLT, H * QD], BF16)
            wk = pg.tile([128, CT, HPC * NOPE], BF16)
            wv = pg.tile([128, CT, HPC * VD], BF16)
            wo = pg.tile([128, HPC, HID], BF16)
            msk = pg.tile([128, 4, SB], F32)

            # --- kv LoRA-A ---
            ckvu = pa.tile([128, CT, SSH], BF16)
            kpe = pa.tile([RP, SSH], BF16)
            p_st = ppst.tile([1, SSH], F32, tag="st", name="cstat")
            sqc = pa.tile([128, CT, SSH], BF16, name="sqc")
            for m in range(CT + 1):
                mw = 128 if m < CT else RP
                p_a = ppa.tile([128, SSH], F32, tag="a")
                for k in range(KT):
                    nc.tensor.matmul(p_a[:mw, :], wkva[:, k, m * 128:m * 128 + mw],
                                     xt[:, k, :], start=(k == 0), stop=(k == KT - 1))
                if m < CT:
                    nc.vector.tensor_copy(ckvu[:, m, :], p_a[:])
                    nc.vector.tensor_mul(sqc[:, m, :], ckvu[:, m, :], ckvu[:, m, :])
                else:
                    nc.vector.tensor_copy(kpe[:], p_a[:mw, :])
            for m in range(CT):
                nc.tensor.matmul(p_st[:], ones_c[:], sqc[:, m, :],
                                 start=(m == 0), stop=(m == CT - 1))
            rms_c = pa.tile([1, SSH], BF16)
            nc.scalar.activation(rms_c[:], p_st[:], Sqrt, scale=1.0 / KVL,
                                 bias=eps1[:])
            p_bc = ppm.tile([128, SSH], F32, tag="m")
            nc.tensor.matmul(p_bc[:], ones_r[:], rms_c[:], start=True, stop=True)
            invc = pa.tile([128, SSH], BF16)
            nc.vector.reciprocal(invc[:], p_bc[:])
            ckv = pa.tile([128, CT, SSH], BF16)
            for m in range(CT):
                nc.vector.tensor_mul(ckv[:, m, :], ckvu[:, m, :], invc[:])
            nc.scalar.dma_start(
                out=ag_in[0:KVL, :].rearrange("(t p) c -> p t c", p=128),
                in_=ckv[:])
            # --- k_pe rope (scale folded: none needed in bf16) ---
            p_rk = ppm.tile([128, SSH], F32, tag="m", name="rotk")
            nc.tensor.matmul(p_rk[:RP, :], rotq[0:RP, 0:RP], kpe[:],
                             start=True, stop=True)
            rk16 = pas.tile([RP, SSH], BF16, tag="rk")
            nc.vector.tensor_copy(rk16[:], p_rk[:RP, :])
            t1 = pas.tile([RP, SSH], BF16, tag="t1")
            nc.vector.tensor_mul(t1[:], kpe[:], cosd[0:RP, :])
            t2 = pas.tile([RP, SSH], BF16, tag="t2")
            nc.vector.tensor_mul(t2[:], rk16[:], sind[0:RP, :])
            kpd = pa.tile([RP, SSH], BF16)
            nc.vector.tensor_add(kpd[:], t1[:], t2[:])
            nc.scalar.dma_start(out=ag_in[KVL:KVL + RP, :], in_=kpd[:])
            # --- collective #1: AllGather latent+kpe ---
            nc.gpsimd.collective_compute(
                "AllGather", mybir.AluOpType.bypass, replica_groups=groups,
                ins=[ag_in[:].opt()], outs=[ag_out[:].opt()])
            # deferred bulk weight loads, chained with write-after-read
            # gates so each transfer enters the exclusive DMA FIFO after the
            # AllGather staging and after the previous weight transfer.
            agmark = pc.tile([1, 2], BF16, name="agmark")
            nc.gpsimd.dma_start(out=agmark[0:1, 0:2],
                                in_=ag_in[KVL + RP - 1:KVL + RP, 0:2])
            Mul = mybir.AluOpType.mult

            def gate_read(region, token):
                g = pas.tile([1, 1], BF16, tag="g8")
                nc.vector.tensor_tensor(g[:], region, token, Mul)

            gate_read(wqa[0:1, 0, QL - 1:QL], ckv[0:1, CT - 1, 0:1])
            nc.gpsimd.dma_start(out=wqa[:, :, QL // 2:QL],
                                in_=d_wqa[:, :, QL // 2:QL])
            gate_read(wqb[0:1, 0, H * QD - 1:H * QD], agmark[0:1, 0:1])
            nc.gpsimd.dma_start(out=wqb[:, :, H * NOPE:H * QD],
                                in_=d_wqb[:, :, H * NOPE:H * QD])
            for q4 in range(4):
                qs = slice(q4 * H * NOPE // 4, (q4 + 1) * H * NOPE // 4)
                gate_read(wqb[0:1, 0, q4 * H * NOPE // 4:q4 * H * NOPE // 4 + 1],
                          wqb[0:1, 0, H * QD - 1:H * QD])
                nc.gpsimd.dma_start(out=wqb[:, :, qs], in_=d_wqb[:, :, qs])
            for wtile, dsrc in ((wk, d_wk), (wv, d_wv), (wo, d_wo), (msk, d_msk)):
                gate_read(wtile[0:1, 0, 0:1], wqb[0:1, 0, H * NOPE - 1:H * NOPE])
                nc.gpsimd.dma_start(out=wtile, in_=dsrc)

            # --- q LoRA-A ---
            qlu = pa.tile([128, QLT, SSH], BF16)
            p_qst = ppst.tile([1, SSH], F32, tag="st", name="qstat")
            sqq = pa.tile([128, QLT, SSH], BF16, name="sqq")
            for k in range(QLT):
                p_a = ppa.tile([128, SSH], F32, tag="a")
                for kk in range(KT):
                    nc.tensor.matmul(p_a[:], wqa[:, kk, k * 128:(k + 1) * 128],
                                     xt[:, kk, :], start=(kk == 0), stop=(kk == KT - 1))
                nc.vector.tensor_copy(qlu[:, k, :], p_a[:])
                nc.vector.tensor_mul(sqq[:, k, :], qlu[:, k, :], qlu[:, k, :])
            for k in range(QLT):
                nc.tensor.matmul(p_qst[:], ones_c[:], sqq[:, k, :],
                                 start=(k == 0), stop=(k == QLT - 1))
            rms_q = pa.tile([1, SSH], BF16)
            nc.scalar.activation(rms_q[:], p_qst[:], Sqrt, scale=1.0 / QL,
                                 bias=eps1[:])
            p_bq = ppm.tile([128, SSH], F32, tag="m")
            nc.tensor.matmul(p_bq[:], ones_r[:], rms_q[:], start=True, stop=True)
            invq = pa.tile([128, SSH], F32)
            nc.vector.reciprocal(invq[:], p_bq[:])

            # --- q_b for all heads: rope tiles (16..23) first so the rope
            # chain and the AllToAll staging DMAs start as early as possible;
            # nope tiles follow in parity order (A2A#1's inputs first).
            q16 = pa.tile([128, H + NC, SSH], BF16, name="q16")

            def qb_group(mt):
                p_q = ppa.tile([128, SSH], F32, tag="a")
                for k in range(QLT):
                    nc.tensor.matmul(p_q[:], wqb[:, k, mt * 128:(mt + 1) * 128],
                                     qlu[:, k, :], start=(k == 0), stop=(k == QLT - 1))
                nc.vector.tensor_mul(q16[:, mt, :], p_q[:], invq[:])

            for mt in range(H, H + NC):
                qb_group(mt)
            # rope rotate-half + cos/sin (inputs ready; no PE stalls)
            for d in range(NC):
                p_rq = ppm.tile([128, SSH], F32, tag="m")
                nc.tensor.matmul(p_rq[:], rotq[:], q16[:, H + d, :],
                                 start=True, stop=True)
                rq16 = pas.tile([128, SSH], BF16, tag="rk", name="rq16")
                nc.vector.tensor_copy(rq16[:], p_rq[:])
                t1q = pas.tile([128, SSH], BF16, tag="t1")
                nc.vector.tensor_mul(t1q[:], q16[:, H + d, :], cosd[:])
                t2q = pas.tile([128, SSH], BF16, tag="t2")
                nc.vector.tensor_mul(t2q[:], rq16[:], sind[:])
                nc.vector.tensor_add(q16[:, H + d, :], t1q[:], t2q[:])
            for mt in range(0, H, 2):
                qb_group(mt)
            nc.gpsimd.dma_start(
                out=aa_in[0][:, 0:NOPE, :].rearrange("j p c -> p j c"),
                in_=q16[:, 0:H:2, :].rearrange("p j c -> p j c"))
            nc.gpsimd.dma_start(
                out=aa_in[0][:, NOPE:QD, :].rearrange("j p c -> p j c"),
                in_=q16[0:RP, H:H + NC, :])
            nc.gpsimd.collective_compute(
                "AllToAll", mybir.AluOpType.bypass, replica_groups=groups,
                ins=[aa_in[0][:].opt()], outs=[aa_out[0][:].opt()])
            for mt in range(1, H, 2):
                qb_group(mt)
            nc.gpsimd.dma_start(
                out=aa_in[1][:, 0:NOPE, :].rearrange("j p c -> p j c"),
                in_=q16[:, 1:H:2, :].rearrange("p j c -> p j c"))
            nc.gpsimd.dma_start(
                out=aa_in[1][:, NOPE:QD, :].rearrange("j p c -> p j c"),
                in_=q16[RP:128, H:H + NC, :])
            nc.gpsimd.collective_compute(
                "AllToAll", mybir.AluOpType.bypass, replica_groups=groups,
                ins=[aa_in[1][:].opt()], outs=[aa_out[1][:].opt()])

        # =============== stage B: head-local attention ===============
        with tc.tile_pool(name="pB", bufs=1) as pb, \
             tc.tile_pool(name="pBe", bufs=10) as pbe, \
             tc.tile_pool(name="pBo", bufs=4) as pbo, \
             tc.tile_pool(name="pBn", bufs=4) as pbn, \
             tc.tile_pool(name="ppS", bufs=3, space="PSUM") as pps, \
             tc.tile_pool(name="ppO", bufs=2, space="PSUM") as ppo, \
             tc.tile_pool(name="ppD", bufs=1, space="PSUM") as ppd, \
             tc.tile_pool(name="ppC", bufs=2, space="PSUM") as ppc:
            ckvg = pb.tile([128, CT, S], BF16)
            for t in range(CT):
                nc.gpsimd.dma_start(
                    out=ckvg[:, t, :].rearrange("p (j c) -> p j c", j=NC),
                    in_=ag_out[:, t * 128:(t + 1) * 128, :].rearrange(
                        "j p c -> p j c"))
            kpdg = pb.tile([RP, S], BF16)
            nc.gpsimd.dma_start(out=kpdg[:].rearrange("p (j c) -> p j c", j=NC),
                              in_=ag_out[:, KVL:KVL + RP, :].rearrange(
                                  "j p c -> p j c"))
            qt = [pb.tile([128, S], BF16, name=f"qt{h}") for h in range(HPC)]
            qpt = [pb.tile([RP, S], BF16, name=f"qpt{h}") for h in range(HPC)]

            def unpack_q(h):
                nc.gpsimd.dma_start(
                    out=qt[h][:].rearrange("p (j c) -> p j c", j=NC),
                    in_=aa_out[h][:, 0:NOPE, :].rearrange("j p c -> p j c"))
                nc.gpsimd.dma_start(
                    out=qpt[h][:].rearrange("p (j c) -> p j c", j=NC),
                    in_=aa_out[h][:, NOPE:QD, :].rearrange("j p c -> p j c"))

            # --- kv_b: kn per head, v (both heads) keys-on-partitions ---
            kn = [pb.tile([128, S], BF16, name=f"kn{h}") for h in range(HPC)]
            for h in range(HPC):
                for cb in range(S // SSH):
                    p_k = ppc.tile([128, SSH], F32, tag="c")
                    for t in range(CT):
                        nc.tensor.matmul(p_k[:], wk[:, t, h * NOPE:(h + 1) * NOPE],
                                         ckvg[:, t, cb * SSH:(cb + 1) * SSH],
                                         start=(t == 0), stop=(t == CT - 1))
                    nc.any.tensor_copy(kn[h][:, cb * SSH:(cb + 1) * SSH], p_k[:])
            vst = pb.tile([128, S // 128, HPC * VD], BF16)
            for sb in range(S // 128):
                p_v = ppc.tile([128, HPC * VD], F32, tag="c")
                for t in range(CT):
                    nc.tensor.matmul(p_v[:], ckvg[:, t, sb * 128:(sb + 1) * 128],
                                     wv[:, t, :], start=(t == 0), stop=(t == CT - 1))
                nc.any.tensor_copy(vst[:, sb, :], p_v[:])

            # --- attention: heads outer (matches AllToAll arrival).
            # Software-pipelined: AV/den for ik are issued after the scores
            # of ik+1 so the PE never stalls on the exp; the per-(qb,h)
            # normalization finisher is deferred into the next iteration's
            # matmul stream.
            ao = pb.tile([128, NSB, HPC, SB], BF16)
            pending = None

            def finisher(fin):
                h, qb, p_o, p_d = fin
                den = pbn.tile([1, SB], BF16, tag="den")
                nc.vector.tensor_copy(den[:], p_d[:])
                p_b = ppc.tile([128, SB], F32, tag="c", name="bcast")
                nc.tensor.matmul(p_b[:], ones_r[:], den[:], start=True, stop=True)
                rec = pbn.tile([128, SB], F32, tag="rec")
                nc.vector.reciprocal(rec[:], p_b[:])
                nc.vector.tensor_mul(ao[:, qb, h, :], p_o[:], rec[:])

            def oproj(qb):
                for st in range(SB // 128):
                    sc = slice(qb * SB + st * 128, qb * SB + (st + 1) * 128)
                    ot = pbo.tile([128, HID], BF16, tag="ot")
                    for nb in range(HID // SB):
                        ncols = bass.ts(nb, SB)
                        p_c = ppc.tile([128, SB], F32, tag="c")
                        for hh in range(HPC):
                            nc.tensor.matmul(
                                p_c[:], ao[:, qb, hh, st * 128:(st + 1) * 128],
                                wo[:, hh, ncols],
                                start=(hh == 0), stop=(hh == HPC - 1))
                        nc.vector.tensor_copy(ot[:, ncols], p_c[:])
                    nc.sync.dma_start(out=d_out[sc, :], in_=ot[:])

            for h in range(HPC):
                unpack_q(h)
                for qb in range(NSB):
                    qcols = bass.ts(qb, SB)
                    nk = 4 * (qb + 1)
                    p_o = ppo.tile([128, SB], F32, tag="o")
                    p_d = ppd.tile([1, SB], F32, tag="d")
                    ework = []

                    def av_den(pik, pe_):
                        nc.tensor.matmul(p_o[:], vst[:, pik, h * VD:(h + 1) * VD],
                                         pe_[:], start=(pik == 0),
                                         stop=(pik == nk - 1))
                        nc.tensor.matmul(p_d[:], ones_c[:], pe_[:],
                                         start=(pik == 0), stop=(pik == nk - 1))

                    for ik in range(nk):
                        kc = slice(ik * 128, (ik + 1) * 128)
                        p_s = pps.tile([128, SB], F32, tag="s")
                        nc.tensor.matmul(p_s[:], kn[h][:, kc], qt[h][:, qcols],
                                         start=True, stop=False)
                        nc.tensor.matmul(p_s[:], kpdg[:, kc], qpt[h][:, qcols],
                                         start=False, stop=True)
                        if ik == 2 and pending is not None:
                            fin, oqb = pending
                            finisher(fin)
                            pending = None
                            if oqb is not None:
                                oproj(oqb)
                        if len(ework) == 2:
                            av_den(*ework.pop(0))
                        r = ik - 4 * qb
                        if r >= 0:
                            nc.vector.tensor_add(p_s[:], p_s[:], msk[:, r, :])
                        e = pbe.tile([128, SB], BF16, tag="e")
                        nc.scalar.activation(e[:], p_s[:], Exp, scale=SCALE)
                        ework.append((ik, e))
                    for item in ework:
                        av_den(*item)
                    pending = ((h, qb, p_o, p_d),
                               qb if h == HPC - 1 else None)
            fin, oqb = pending
            finisher(fin)
            if oqb is not None:
                oproj(oqb)


def _host_constants():
    inv_freq = 1.0 / (ROPE_THETA ** (np.arange(0, RP, 2, dtype=np.float32) / RP))
    t = np.arange(S, dtype=np.float32)
    freqs = np.outer(t, inv_freq)
    emb = np.concatenate([freqs, freqs], -1)          # [S, 64]
    cos, sin = np.cos(emb), np.sin(emb)
    cosd = np.concatenate([cos.T, cos.T], 0).astype(np.float32)   # [128, S]
    sind = np.concatenate([sin.T, sin.T], 0).astype(np.float32)

    # additive causal mask for diagonal 128-key blocks: [128, 4, 512]
    mska = np.zeros((128, 4, SB), np.float32)
    for r in range(4):
        for p in range(128):
            mska[p, r, :p + 128 * r] = NEG
    # rotate-half as matmul lhsT: same as baseline
    Q = np.zeros((RP, RP), np.float32)
    for i in range(RP // 2):
        Q[i, i + RP // 2] = -1.0
        Q[i + RP // 2, i] = 1.0
    P = np.zeros((128, 128), np.float32)
    P[:RP, :RP] = Q
    P[RP:, RP:] = Q
    rotq = P.T.copy()
    return cosd, sind, mska, rotq


def _tile3(w, kt):
    """[kt*128, F] -> [128, kt, F]"""
    return np.ascontiguousarray(
        w.reshape(kt, 128, w.shape[1]).transpose(1, 0, 2))


def kernel(hidden_states, w_q_a, q_a_weight, w_q_b, w_kv_a, kv_a_weight,
           w_kv_b, w_o):
    global LAST_RESULT
    if "nc" not in _CACHE:
        _CACHE["nc"] = _build_program()
    nc = _CACHE["nc"]

    x = np.asarray(hidden_states, np.float32)[0]       # [S, 2048]
    xt = np.ascontiguousarray(x.T)                     # [2048, S]
    wqa_t = np.asarray(w_q_a, np.float32).T            # [HID, QL]
    wkva_t = np.asarray(w_kv_a, np.float32).T          # [HID, KVL+RP]
    wqb_eff = np.asarray(w_q_b, np.float32) * np.asarray(q_a_weight, np.float32)[None, :]
    wkvb_eff = np.asarray(w_kv_b, np.float32) * np.asarray(kv_a_weight, np.float32)[None, :]
    won = np.asarray(w_o, np.float32)                  # [HID, H*VD]

    # q_b output feature permutation: nope head-major, then rope packed 2/tile
    perm = np.zeros(H * QD, np.int64)
    for h in range(H):
        perm[h * NOPE:(h + 1) * NOPE] = h * QD + np.arange(NOPE)
    base = H * NOPE
    for d in range(NC):
        for j in range(HPC):
            hh = 2 * d + j
            perm[base + d * 128 + j * RP: base + d * 128 + (j + 1) * RP] = \
                hh * QD + NOPE + np.arange(RP)
    wqb_p = wqb_eff[perm, :]                           # [3072, QL]

    cosd, sind, mska, rotq = _host_constants()

    wqa16 = _tile3(wqa_t, KT).astype(NPBF)
    wkva16 = _tile3(wkva_t, KT).astype(NPBF)
    wqb16 = _tile3(np.ascontiguousarray(wqb_p.T), QLT).astype(NPBF)
    rotq16 = rotq.astype(NPBF)

    shared = {"wqa16": wqa16, "wkva16": wkva16, "wqb16": wqb16,
              "maskadd": mska, "rotq16": rotq16}

    in_maps = []
    for c in range(NC):
        h0, h1 = HPC * c, HPC * c + 1
        wk_t = np.concatenate(
            [wkvb_eff[h * (NOPE + VD):h * (NOPE + VD) + NOPE] for h in (h0, h1)],
            0).T                                        # [KVL, 256]
        wv_t = np.concatenate(
            [wkvb_eff[h * (NOPE + VD) + NOPE:(h + 1) * (NOPE + VD)] for h in (h0, h1)],
            0).T                                        # [KVL, 256]
        wo_t = np.stack(
            [np.ascontiguousarray(won[:, h * VD:(h + 1) * VD].T) for h in (h0, h1)],
            1)                                          # [128, 2, HID]
        cols = slice(c * SSH, (c + 1) * SSH)
        im = dict(shared)
        im.update({
            "xt16": _tile3(np.ascontiguousarray(xt[:, cols]), KT).astype(NPBF),
            "wk16": _tile3(wk_t, CT).astype(NPBF),
            "wv16": _tile3(wv_t, CT).astype(NPBF),
            "wo16": np.ascontiguousarray(wo_t).astype(NPBF),
            "cosd": np.ascontiguousarray(cosd[:, cols]).astype(NPBF),
            "sind": np.ascontiguousarray(sind[:, cols]).astype(NPBF),
        })
        in_maps.append(im)

    res = run_bass_kernel_spmd(nc, in_maps, list(range(NC)))
    LAST_RESULT = res
    out = np.zeros((S, HID), np.float32)
    for c in range(NC):
        out += np.asarray(res.results[c]["out"]).astype(np.float32)
    return out.reshape(1, S, HID)


# revision 56
# speedup vs baseline: 1.0063x; 1.0063x over previous
"""MLA (DeepSeek-style multi-head latent attention) forward on 8 trn2 cores.

Layout v2: sequence-sharded LoRA-A + device collectives + bf16 matmuls.

Each core computes the LoRA-A projections (q_latent, compressed-kv latent,
k_pe) only for its 256-column sequence shard (8x less replicated work than
pure head-TP). The normalized kv latent + rope'd k_pe are AllGathered
(shared by every head); the per-head q vectors are redistributed with two
AllToAlls (one per head of each core's head pair) so attention runs fully
head-local: core c owns heads 2c, 2c+1 over the full sequence. kv_b expands
kn/v from the gathered latent per head; o_proj is input-split on heads and
the partial products are summed on the host (the unshard step).

All matmuls run in bf16 (1 PE cycle/row regardless of free-dim size, half
the DMA/communication bytes of fp32; final accuracy ~4e-3 vs the 2e-2
gate). Softmax runs over the key (partition) axis: exp on the scalar
engine, denominator via a ones-column matmul, broadcast of per-column
scalars via a K=1 matmul. RoPE rotate-half is a matmul against a constant
signed permutation. o_proj results are DMA'd directly from PSUM.
"""
import numpy as np
import ml_dtypes

import concourse.bass as bass
import concourse.tile as tile
from concourse import bacc, mybir
from concourse.bass_utils import run_bass_kernel_spmd

F32 = mybir.dt.float32
BF16 = mybir.dt.bfloat16
NPBF = ml_dtypes.bfloat16

HID = 2048
S = 2048
H = 16
QL = 1536
KVL = 512
NOPE = 128
RP = 64
VD = 128
QD = NOPE + RP              # 192
SCALE = QD ** -0.5
EPS = 1e-6
ROPE_THETA = 10000.0

NC = 8
HPC = 2                     # heads per core
SSH = S // NC               # 256-seq shard
KT = HID // 128             # 16
QLT = QL // 128             # 12
CT = KVL // 128             # 4
SB = 512                    # attention query block
NSB = S // SB               # 4
NEG = -30000.0

_CACHE = {}
LAST_RESULT = None


def _build_program():
    nc = bacc.Bacc("TRN2", target_bir_lowering=False, debug=False,
                   num_devices=NC)
    d_xt = nc.dram_tensor("xt16", [128, KT, SSH], BF16, kind="ExternalInput").ap()
    d_wqa = nc.dram_tensor("wqa16", [128, KT, QL], BF16, kind="ExternalInput").ap()
    d_wkva = nc.dram_tensor("wkva16", [128, KT, KVL + RP], BF16, kind="ExternalInput").ap()
    d_wqb = nc.dram_tensor("wqb16", [128, QLT, H * QD], BF16, kind="ExternalInput").ap()
    d_wk = nc.dram_tensor("wk16", [128, CT, HPC * NOPE], BF16, kind="ExternalInput").ap()
    d_wv = nc.dram_tensor("wv16", [128, CT, HPC * VD], BF16, kind="ExternalInput").ap()
    d_wo = nc.dram_tensor("wo16", [128, HPC, HID], BF16, kind="ExternalInput").ap()
    d_cos = nc.dram_tensor("cosd", [128, SSH], BF16, kind="ExternalInput").ap()
    d_sin = nc.dram_tensor("sind", [128, SSH], BF16, kind="ExternalInput").ap()
    d_msk = nc.dram_tensor("maskadd", [128, 4, SB], F32, kind="ExternalInput").ap()
    d_rotq = nc.dram_tensor("rotq16", [128, 128], BF16, kind="ExternalInput").ap()
    d_out = nc.dram_tensor("out", [S, HID], BF16, kind="ExternalOutput").ap()

    with tile.TileContext(nc) as tc:
        _mla(tc, d_xt, d_wqa, d_wkva, d_wqb, d_wk, d_wv, d_wo, d_cos, d_sin,
             d_msk, d_rotq, d_out)
    nc.compile()
    return nc


def _mla(tc, d_xt, d_wqa, d_wkva, d_wqb, d_wk, d_wv, d_wo, d_cos, d_sin,
         d_msk, d_rotq, d_out):
    nc = tc.nc
    Exp = mybir.ActivationFunctionType.Exp
    Sqrt = mybir.ActivationFunctionType.Sqrt
    groups = [list(range(NC))]

    with nc.allow_low_precision(reason="bf16 pipeline"), \
         tc.tile_pool(name="pdram", bufs=1, space="DRAM") as pdram, \
         tc.tile_pool(name="pconst", bufs=1) as pc, \
         tc.tile_pool(name="pglob", bufs=1) as pg:
        # ---- DRAM bounce buffers for collectives ----
        ag_in = pdram.tile([KVL + RP, SSH], BF16)
        ag_out = pdram.tile([NC, KVL + RP, SSH], BF16)
        aa_in = [pdram.tile([NC, QD, SSH], BF16, name=f"aain{i}") for i in range(HPC)]
        aa_out = [pdram.tile([NC, QD, SSH], BF16, name=f"aaout{i}") for i in range(HPC)]

        # ---- small constants ----
        ones_c = pc.tile([128, 1], BF16)
        nc.vector.memset(ones_c, 1.0)
        ones_r = pc.tile([1, 128], BF16)
        nc.vector.memset(ones_r, 1.0)
        eps1 = pc.tile([1, 1], F32)
        nc.vector.memset(eps1, EPS)
        warm = pc.tile([1, 2], F32, name="actwarm")
        nc.scalar.activation(warm[0:1, 0:1], eps1[:], Sqrt)
        nc.scalar.activation(warm[0:1, 1:2], eps1[:], Exp)
        rotq = pc.tile([128, 128], BF16)
        nc.sync.dma_start(out=rotq, in_=d_rotq)
        cosd = pc.tile([128, SSH], BF16)
        nc.sync.dma_start(out=cosd, in_=d_cos)
        sind = pc.tile([128, SSH], BF16)
        nc.sync.dma_start(out=sind, in_=d_sin)

        # =============== stage A: shard projections ===============
        with tc.tile_pool(name="pw", bufs=1) as pw, \
             tc.tile_pool(name="pA", bufs=1) as pa, \
             tc.tile_pool(name="pAq", bufs=3) as paq, \
             tc.tile_pool(name="pAs", bufs=3) as pas, \
             tc.tile_pool(name="ppA", bufs=3, space="PSUM") as ppa, \
             tc.tile_pool(name="ppSt", bufs=2, space="PSUM") as ppst, \
             tc.tile_pool(name="ppM", bufs=2, space="PSUM") as ppm:
            # ---- stage-A weights: per-k-slice tiles so matmuls start early ----
            xt = pw.tile([128, KT, SSH], BF16)
            # Early (eager) loads on the SP HW queue: only what the first
            # ~20us of compute needs. Everything else is loaded via gpsimd
            # SWDGE triggers placed AFTER the AllGather in program order, so
            # those transfers enter the global DMA FIFO behind the
            # collective staging instead of ahead of it.
            wkva = pw.tile([128, KT, KVL + RP], BF16)
            for half in range(2):
                hk = slice(half * KT // 2, (half + 1) * KT // 2)
                nc.sync.dma_start(out=xt[:, hk, :], in_=d_xt[:, hk, :])
                nc.sync.dma_start(out=wkva[:, hk, :], in_=d_wkva[:, hk, :])
            wqa = pw.tile([128, KT, QL], BF16)
            # gate wqa-colA behind wkva via a write-after-read dep: the
            # reader consumes both the colA region and the wkva tail, so the
            # colA DMA (a writer of that region) must wait for wkva.
            gate = pc.tile([1, 2], BF16, name="gate")
            nc.vector.tensor_tensor(gate[0:1, 0:1], wqa[0:1, 0, 0:1],
                                    wkva[0:1, KT - 1, 0:1], mybir.AluOpType.mult)
            nc.scalar.dma_start(out=wqa[:, :, 0:QL // 2], in_=d_wqa[:, :, 0:QL // 2])
            wqb = pw.tile([128, QLT, H * QD], BF16)
            wk = pg.tile([128, CT, HPC * NOPE], BF16)
            wv = pg.tile([128, CT, HPC * VD], BF16)
            wo = pg.tile([128, HPC, HID], BF16)
            msk = pg.tile([128, 4, SB], F32)

            # --- kv LoRA-A ---
            ckvu = pa.tile([128, CT, SSH], BF16)
            kpe = pa.tile([RP, SSH], BF16)
            p_st = ppst.tile([1, SSH], F32, tag="st", name="cstat")
            sqc = pa.tile([128, CT, SSH], BF16, name="sqc")
            for m in range(CT + 1):
                mw = 128 if m < CT else RP
                p_a = ppa.tile([128, SSH], F32, tag="a")
                for k in range(KT):
                    nc.tensor.matmul(p_a[:mw, :], wkva[:, k, m * 128:m * 128 + mw],
                                     xt[:, k, :], start=(k == 0), stop=(k == KT - 1))
                if m < CT:
                    nc.vector.tensor_copy(ckvu[:, m, :], p_a[:])
                    nc.vector.tensor_mul(sqc[:, m, :], ckvu[:, m, :], ckvu[:, m, :])
                else:
                    nc.vector.tensor_copy(kpe[:], p_a[:mw, :])
            for m in range(CT):
                nc.tensor.matmul(p_st[:], ones_c[:], sqc[:, m, :],
                                 start=(m == 0), stop=(m == CT - 1))
            rms_c = pa.tile([1, SSH], BF16)
            nc.scalar.activation(rms_c[:], p_st[:], Sqrt, scale=1.0 / KVL,
                                 bias=eps1[:])
            p_bc = ppm.tile([128, SSH], F32, tag="m")
            nc.tensor.matmul(p_bc[:], ones_r[:], rms_c[:], start=True, stop=True)
            invc = pa.tile([128, SSH], BF16)
            nc.vector.reciprocal(invc[:], p_bc[:])
            ckv = pa.tile([128, CT, SSH], BF16)
            for m in range(CT):
                nc.vector.tensor_mul(ckv[:, m, :], ckvu[:, m, :], invc[:])
            nc.scalar.dma_start(
                out=ag_in[0:KVL, :].rearrange("(t p) c -> p t c", p=128),
                in_=ckv[:])
            # --- k_pe rope (scale folded: none needed in bf16) ---
            p_rk = ppm.tile([128, SSH], F32, tag="m", name="rotk")
            nc.tensor.matmul(p_rk[:RP, :], rotq[0:RP, 0:RP], kpe[:],
                             start=True, stop=True)
            rk16 = pas.tile([RP, SSH], BF16, tag="rk")
            nc.vector.tensor_copy(rk16[:], p_rk[:RP, :])
            t1 = pas.tile([RP, SSH], BF16, tag="t1")
            nc.vector.tensor_mul(t1[:], kpe[:], cosd[0:RP, :])
            t2 = pas.tile([RP, SSH], BF16, tag="t2")
            nc.vector.tensor_mul(t2[:], rk16[:], sind[0:RP, :])
            kpd = pa.tile([RP, SSH], BF16)
            nc.vector.tensor_add(kpd[:], t1[:], t2[:])
            nc.scalar.dma_start(out=ag_in[KVL:KVL + RP, :], in_=kpd[:])
            # --- collective #1: AllGather latent+kpe ---
            nc.gpsimd.collective_compute(
                "AllGather", mybir.AluOpType.bypass, replica_groups=groups,
                ins=[ag_in[:].opt()], outs=[ag_out[:].opt()])
            # deferred bulk weight loads, chained with write-after-read
            # gates so each transfer enters the exclusive DMA FIFO after the
            # AllGather staging and after the previous weight transfer.
            agmark = pc.tile([1, 2], BF16, name="agmark")
            nc.gpsimd.dma_start(out=agmark[0:1, 0:2],
                                in_=ag_in[KVL + RP - 1:KVL + RP, 0:2])
            Mul = mybir.AluOpType.mult

            def gate_read(region, token):
                g = pas.tile([1, 1], BF16, tag="g8")
                nc.vector.tensor_tensor(g[:], region, token, Mul)

            gate_read(wqa[0:1, 0, QL - 1:QL], ckv[0:1, CT - 1, 0:1])
            nc.gpsimd.dma_start(out=wqa[:, :, QL // 2:QL],
                                in_=d_wqa[:, :, QL // 2:QL])
            gate_read(wqb[0:1, 0, H * QD - 1:H * QD], agmark[0:1, 0:1])
            nc.gpsimd.dma_start(out=wqb[:, :, H * NOPE:H * QD],
                                in_=d_wqb[:, :, H * NOPE:H * QD])
            for q4 in range(4):
                qs = slice(q4 * H * NOPE // 4, (q4 + 1) * H * NOPE // 4)
                gate_read(wqb[0:1, 0, q4 * H * NOPE // 4:q4 * H * NOPE // 4 + 1],
                          wqb[0:1, 0, H * QD - 1:H * QD])
                nc.gpsimd.dma_start(out=wqb[:, :, qs], in_=d_wqb[:, :, qs])
            for wtile, dsrc in ((wk, d_wk), (wv, d_wv), (wo, d_wo), (msk, d_msk)):
                gate_read(wtile[0:1, 0, 0:1], wqb[0:1, 0, H * NOPE - 1:H * NOPE])
                nc.gpsimd.dma_start(out=wtile, in_=dsrc)

            # --- q LoRA-A ---
            qlu = pa.tile([128, QLT, SSH], BF16)
            p_qst = ppst.tile([1, SSH], F32, tag="st", name="qstat")
            sqq = pa.tile([128, QLT, SSH], BF16, name="sqq")
            for k in range(QLT):
                p_a = ppa.tile([128, SSH], F32, tag="a")
                for kk in range(KT):
                    nc.tensor.matmul(p_a[:], wqa[:, kk, k * 128:(k + 1) * 128],
                                     xt[:, kk, :], start=(kk == 0), stop=(kk == KT - 1))
                nc.vector.tensor_copy(qlu[:, k, :], p_a[:])
                nc.vector.tensor_mul(sqq[:, k, :], qlu[:, k, :], qlu[:, k, :])
            for k in range(QLT):
                nc.tensor.matmul(p_qst[:], ones_c[:], sqq[:, k, :],
                                 start=(k == 0), stop=(k == QLT - 1))
            rms_q = pa.tile([1, SSH], BF16)
            nc.scalar.activation(rms_q[:], p_qst[:], Sqrt, scale=1.0 / QL,
                                 bias=eps1[:])
            p_bq = ppm.tile([128, SSH], F32, tag="m")
            nc.tensor.matmul(p_bq[:], ones_r[:], rms_q[:], start=True, stop=True)
            invq = pa.tile([128, SSH], F32)
            nc.vector.reciprocal(invq[:], p_bq[:])

            # --- q_b for all heads: rope tiles (16..23) first so the rope
            # chain and the AllToAll staging DMAs start as early as possible;
            # nope tiles follow in parity order (A2A#1's inputs first).
            q16 = pa.tile([128, H + NC, SSH], BF16, name="q16")

            def qb_group(mt):
                p_q = ppa.tile([128, SSH], F32, tag="a")
                for k in range(QLT):
                    nc.tensor.matmul(p_q[:], wqb[:, k, mt * 128:(mt + 1) * 128],
                                     qlu[:, k, :], start=(k == 0), stop=(k == QLT - 1))
                nc.vector.tensor_mul(q16[:, mt, :], p_q[:], invq[:])

            for mt in range(H, H + NC):
                qb_group(mt)
            # rope rotate-half + cos/sin (inputs ready; no PE stalls)
            for d in range(NC):
                p_rq = ppm.tile([128, SSH], F32, tag="m")
                nc.tensor.matmul(p_rq[:], rotq[:], q16[:, H + d, :],
                                 start=True, stop=True)
                rq16 = pas.tile([128, SSH], BF16, tag="rk", name="rq16")
                nc.vector.tensor_copy(rq16[:], p_rq[:])
                t1q = pas.tile([128, SSH], BF16, tag="t1")
                nc.vector.tensor_mul(t1q[:], q16[:, H + d, :], cosd[:])
                t2q = pas.tile([128, SSH], BF16, tag="t2")
                nc.vector.tensor_mul(t2q[:], rq16[:], sind[:])
                nc.vector.tensor_add(q16[:, H + d, :], t1q[:], t2q[:])
            for mt in range(0, H, 2):
                qb_group(mt)
            nc.gpsimd.dma_start(
                out=aa_in[0][:, 0:NOPE, :].rearrange("j p c -> p j c"),
                in_=q16[:, 0:H:2, :].rearrange("p j c -> p j c"))
            nc.gpsimd.dma_start(
                out=aa_in[0][:, NOPE:QD, :].rearrange("j p c -> p j c"),
                in_=q16[0:RP, H:H + NC, :])
            nc.gpsimd.collective_compute(
                "AllToAll", mybir.AluOpType.bypass, replica_groups=groups,
                ins=[aa_in[0][:].opt()], outs=[aa_out[0][:].opt()])
            for mt in range(1, H, 2):
                qb_group(mt)
            nc.gpsimd.dma_start(
                out=aa_in[1][:, 0:NOPE, :].rearrange("j p c -> p j c"),
                in_=q16[:, 1:H:2, :].rearrange("p j c -> p j c"))
            nc.gpsimd.dma_start(
                out=aa_in[1][:, NOPE:QD, :].rearrange("j p c -> p j c"),
                in_=q16[RP:128, H:H + NC, :])
            nc.gpsimd.collective_compute(
                "AllToAll", mybir.AluOpType.bypass, replica_groups=groups,
                ins=[aa_in[1][:].opt()], outs=[aa_out[1][:].opt()])

        # =============== stage B: head-local attention ===============
        with tc.tile_pool(name="pB", bufs=1) as pb, \
             tc.tile_pool(name="pBe", bufs=10) as pbe, \
             tc.tile_pool(name="pBo", bufs=4) as pbo, \
             tc.tile_pool(name="pBn", bufs=4) as pbn, \
             tc.tile_pool(name="ppS", bufs=3, space="PSUM") as pps, \
             tc.tile_pool(name="ppO", bufs=2, space="PSUM") as ppo, \
             tc.tile_pool(name="ppD", bufs=1, space="PSUM") as ppd, \
             tc.tile_pool(name="ppC", bufs=2, space="PSUM") as ppc:
            ckvg = pb.tile([128, CT, S], BF16)
            for t in range(CT):
                nc.gpsimd.dma_start(
                    out=ckvg[:, t, :].rearrange("p (j c) -> p j c", j=NC),
                    in_=ag_out[:, t * 128:(t + 1) * 128, :].rearrange(
                        "j p c -> p j c"))
            kpdg = pb.tile([RP, S], BF16)
            nc.gpsimd.dma_start(out=kpdg[:].rearrange("p (j c) -> p j c", j=NC),
                              in_=ag_out[:, KVL:KVL + RP, :].rearrange(
                                  "j p c -> p j c"))
            qt = [pb.tile([128, S], BF16, name=f"qt{h}") for h in range(HPC)]
            qpt = [pb.tile([RP, S], BF16, name=f"qpt{h}") for h in range(HPC)]

            def unpack_q(h):
                nc.gpsimd.dma_start(
                    out=qt[h][:].rearrange("p (j c) -> p j c", j=NC),
                    in_=aa_out[h][:, 0:NOPE, :].rearrange("j p c -> p j c"))
                nc.gpsimd.dma_start(
                    out=qpt[h][:].rearrange("p (j c) -> p j c", j=NC),
                    in_=aa_out[h][:, NOPE:QD, :].rearrange("j p c -> p j c"))

            # --- kv_b: kn per head, v (both heads) keys-on-partitions ---
            kn = [pb.tile([128, S], BF16, name=f"kn{h}") for h in range(HPC)]
            for h in range(HPC):
                for cb in range(S // SSH):
                    p_k = ppc.tile([128, SSH], F32, tag="c")
                    for t in range(CT):
                        nc.tensor.matmul(p_k[:], wk[:, t, h * NOPE:(h + 1) * NOPE],
                                         ckvg[:, t, cb * SSH:(cb + 1) * SSH],
                                         start=(t == 0), stop=(t == CT - 1))
                    nc.any.tensor_copy(kn[h][:, cb * SSH:(cb + 1) * SSH], p_k[:])
            vst = pb.tile([128, S // 128, HPC * VD], BF16)
            for sb in range(S // 128):
                p_v = ppc.tile([128, HPC * VD], F32, tag="c")
                for t in range(CT):
                    nc.tensor.matmul(p_v[:], ckvg[:, t, sb * 128:(sb + 1) * 128],
                                     wv[:, t, :], start=(t == 0), stop=(t == CT - 1))
                nc.any.tensor_copy(vst[:, sb, :], p_v[:])

            # --- attention: heads outer (matches AllToAll arrival).
            # Software-pipelined: AV/den for ik are issued after the scores
            # of ik+1 so the PE never stalls on the exp; the per-(qb,h)
            # normalization finisher is deferred into the next iteration's
            # matmul stream.
            ao = pb.tile([128, NSB, HPC, SB], BF16)
            pending = None

            def finisher(fin):
                h, qb, p_o, p_d = fin
                den = pbn.tile([1, SB], BF16, tag="den")
                nc.vector.tensor_copy(den[:], p_d[:])
                p_b = ppc.tile([128, SB], F32, tag="c", name="bcast")
                nc.tensor.matmul(p_b[:], ones_r[:], den[:], start=True, stop=True)
                rec = pbn.tile([128, SB], F32, tag="rec")
                nc.vector.reciprocal(rec[:], p_b[:])
                nc.vector.tensor_mul(ao[:, qb, h, :], p_o[:], rec[:])

            def oproj(qb):
                for st in range(SB // 128):
                    sc = slice(qb * SB + st * 128, qb * SB + (st + 1) * 128)
                    ot = pbo.tile([128, HID], BF16, tag="ot")
                    for nb in range(HID // SB):
                        ncols = bass.ts(nb, SB)
                        p_c = ppc.tile([128, SB], F32, tag="c")
                        for hh in range(HPC):
                            nc.tensor.matmul(
                                p_c[:], ao[:, qb, hh, st * 128:(st + 1) * 128],
                                wo[:, hh, ncols],
                                start=(hh == 0), stop=(hh == HPC - 1))
                        nc.vector.tensor_copy(ot[:, ncols], p_c[:])
                    nc.sync.dma_start(out=d_out[sc, :], in_=ot[:])

            for h in range(HPC):
                unpack_q(h)
                for qb in range(NSB):
                    qcols = bass.ts(qb, SB)
                    nk = 4 * (qb + 1)
                    p_o = ppo.tile([128, SB], F32, tag="o")
                    p_d = ppd.tile([1, SB], F32, tag="d")
                    ework = []

                    def av_den(pik, pe_):
                        nc.tensor.matmul(p_o[:], vst[:, pik, h * VD:(h + 1) * VD],
                                         pe_[:], start=(pik == 0),
                                         stop=(pik == nk - 1))
                        nc.tensor.matmul(p_d[:], ones_c[:], pe_[:],
                                         start=(pik == 0), stop=(pik == nk - 1))

                    for ik in range(nk):
                        kc = slice(ik * 128, (ik + 1) * 128)
                        p_s = pps.tile([128, SB], F32, tag="s")
                        nc.tensor.matmul(p_s[:], kn[h][:, kc], qt[h][:, qcols],
                                         start=True, stop=False)
                        nc.tensor.matmul(p_s[:], kpdg[:, kc], qpt[h][:, qcols],
                                         start=False, stop=True)
                        if ik == 3 and pending is not None:
                            fin, oqb = pending
                            finisher(fin)
                            pending = None
                            if oqb is not None:
                                oproj(oqb)
                        if len(ework) == 5:
                            av_den(*ework.pop(0))
                        r = ik - 4 * qb
                        if r >= 0:
                            nc.vector.tensor_add(p_s[:], p_s[:], msk[:, r, :])
                        e = pbe.tile([128, SB], BF16, tag="e")
                        nc.scalar.activation(e[:], p_s[:], Exp, scale=SCALE)
                        ework.append((ik, e))
                    for item in ework:
                        av_den(*item)
                    pending = ((h, qb, p_o, p_d),
                               qb if h == HPC - 1 else None)
            fin, oqb = pending
            finisher(fin)
            if oqb is not None:
                oproj(oqb)


def _host_constants():
    inv_freq = 1.0 / (ROPE_THETA ** (np.arange(0, RP, 2, dtype=np.float32) / RP))
    t = np.arange(S, dtype=np.float32)
    freqs = np.outer(t, inv_freq)
    emb = np.concatenate([freqs, freqs], -1)          # [S, 64]
    cos, sin = np.cos(emb), np.sin(emb)
    cosd = np.concatenate([cos.T, cos.T], 0).astype(np.float32)   # [128, S]
    sind = np.concatenate([sin.T, sin.T], 0).astype(np.float32)

    # additive causal mask for diagonal 128-key blocks: [128, 4, 512]
    mska = np.zeros((128, 4, SB), np.float32)
    for r in range(4):
        for p in range(128):
            mska[p, r, :p + 128 * r] = NEG
    # rotate-half as matmul lhsT: same as baseline
    Q = np.zeros((RP, RP), np.float32)
    for i in range(RP // 2):
        Q[i, i + RP // 2] = -1.0
        Q[i + RP // 2, i] = 1.0
    P = np.zeros((128, 128), np.float32)
    P[:RP, :RP] = Q
    P[RP:, RP:] = Q
    rotq = P.T.copy()
    return cosd, sind, mska, rotq


def _tile3(w, kt):
    """[kt*128, F] -> [128, kt, F]"""
    return np.ascontiguousarray(
        w.reshape(kt, 128, w.shape[1]).transpose(1, 0, 2))


def kernel(hidden_states, w_q_a, q_a_weight, w_q_b, w_kv_a, kv_a_weight,
           w_kv_b, w_o):
    global LAST_RESULT
    if "nc" not in _CACHE:
        _CACHE["nc"] = _build_program()
    nc = _CACHE["nc"]

    x = np.asarray(hidden_states, np.float32)[0]       # [S, 2048]
    xt = np.ascontiguousarray(x.T)                     # [2048, S]
    wqa_t = np.asarray(w_q_a, np.float32).T            # [HID, QL]
    wkva_t = np.asarray(w_kv_a, np.float32).T          # [HID, KVL+RP]
    wqb_eff = np.asarray(w_q_b, np.float32) * np.asarray(q_a_weight, np.float32)[None, :]
    wkvb_eff = np.asarray(w_kv_b, np.float32) * np.asarray(kv_a_weight, np.float32)[None, :]
    won = np.asarray(w_o, np.float32)                  # [HID, H*VD]

    # q_b output feature permutation: nope head-major, then rope packed 2/tile
    perm = np.zeros(H * QD, np.int64)
    for h in range(H):
        perm[h * NOPE:(h + 1) * NOPE] = h * QD + np.arange(NOPE)
    base = H * NOPE
    for d in range(NC):
        for j in range(HPC):
            hh = 2 * d + j
            perm[base + d * 128 + j * RP: base + d * 128 + (j + 1) * RP] = \
                hh * QD + NOPE + np.arange(RP)
    wqb_p = wqb_eff[perm, :]                           # [3072, QL]

    cosd, sind, mska, rotq = _host_constants()

    wqa16 = _tile3(wqa_t, KT).astype(NPBF)
    wkva16 = _tile3(wkva_t, KT).astype(NPBF)
    wqb16 = _tile3(np.ascontiguousarray(wqb_p.T), QLT).astype(NPBF)
    rotq16 = rotq.astype(NPBF)

    shared = {"wqa16": wqa16, "wkva16": wkva16, "wqb16": wqb16,
              "maskadd": mska, "rotq16": rotq16}

    in_maps = []
    for c in range(NC):
        h0, h1 = HPC * c, HPC * c + 1
        wk_t = np.concatenate(
            [wkvb_eff[h * (NOPE + VD):h * (NOPE + VD) + NOPE] for h in (h0, h1)],
            0).T                                        # [KVL, 256]
        wv_t = np.concatenate(
            [wkvb_eff[h * (NOPE + VD) + NOPE:(h + 1) * (NOPE + VD)] for h in (h0, h1)],
            0).T                                        # [KVL, 256]
        wo_t = np.stack(
            [np.ascontiguousarray(won[:, h * VD:(h + 1) * VD].T) for h in (h0, h1)],
            1)                                          # [128, 2, HID]
        cols = slice(c * SSH, (c + 1) * SSH)
        im = dict(shared)
        im.update({
            "xt16": _tile3(np.ascontiguousarray(xt[:, cols]), KT).astype(NPBF),
            "wk16": _tile3(wk_t, CT).astype(NPBF),
            "wv16": _tile3(wv_t, CT).astype(NPBF),
            "wo16": np.ascontiguousarray(wo_t).astype(NPBF),
            "cosd": np.ascontiguousarray(cosd[:, cols]).astype(NPBF),
            "sind": np.ascontiguousarray(sind[:, cols]).astype(NPBF),
        })
        in_maps.append(im)

    res = run_bass_kernel_spmd(nc, in_maps, list(range(NC)))
    LAST_RESULT = res
    out = np.zeros((S, HID), np.float32)
    for c in range(NC):
        out += np.asarray(res.results[c]["out"]).astype(np.float32)
    return out.reshape(1, S, HID)


# revision 61
# speedup vs baseline: 1.0069x; 1.0006x over previous
"""MLA (DeepSeek-style multi-head latent attention) forward on 8 trn2 cores.

Layout v2: sequence-sharded LoRA-A + device collectives + bf16 matmuls.

Each core computes the LoRA-A projections (q_latent, compressed-kv latent,
k_pe) only for its 256-column sequence shard (8x less replicated work than
pure head-TP). The normalized kv latent + rope'd k_pe are AllGathered
(shared by every head); the per-head q vectors are redistributed with two
AllToAlls (one per head of each core's head pair) so attention runs fully
head-local: core c owns heads 2c, 2c+1 over the full sequence. kv_b expands
kn/v from the gathered latent per head; o_proj is input-split on heads and
the partial products are summed on the host (the unshard step).

All matmuls run in bf16 (1 PE cycle/row regardless of free-dim size, half
the DMA/communication bytes of fp32; final accuracy ~4e-3 vs the 2e-2
gate). Softmax runs over the key (partition) axis: exp on the scalar
engine, denominator via a ones-column matmul, broadcast of per-column
scalars via a K=1 matmul. RoPE rotate-half is a matmul against a constant
signed permutation. o_proj results are DMA'd directly from PSUM.
"""
import numpy as np
import ml_dtypes

import concourse.bass as bass
import concourse.tile as tile
from concourse import bacc, mybir
from concourse.bass_utils import run_bass_kernel_spmd

F32 = mybir.dt.float32
BF16 = mybir.dt.bfloat16
NPBF = ml_dtypes.bfloat16

HID = 2048
S = 2048
H = 16
QL = 1536
KVL = 512
NOPE = 128
RP = 64
VD = 128
QD = NOPE + RP              # 192
SCALE = QD ** -0.5
EPS = 1e-6
ROPE_THETA = 10000.0

NC = 8
HPC = 2                     # heads per core
SSH = S // NC               # 256-seq shard
KT = HID // 128             # 16
QLT = QL // 128             # 12
CT = KVL // 128             # 4
SB = 512                    # attention query block
NSB = S // SB               # 4
NEG = -30000.0

_CACHE = {}
LAST_RESULT = None


def _build_program():
    nc = bacc.Bacc("TRN2", target_bir_lowering=False, debug=False,
                   num_devices=NC)
    d_xt = nc.dram_tensor("xt16", [128, KT, SSH], BF16, kind="ExternalInput").ap()
    d_wqa = nc.dram_tensor("wqa16", [128, KT, QL], BF16, kind="ExternalInput").ap()
    d_wkva = nc.dram_tensor("wkva16", [128, KT, KVL + RP], BF16, kind="ExternalInput").ap()
    d_wqb = nc.dram_tensor("wqb16", [128, QLT, H * QD], BF16, kind="ExternalInput").ap()
    d_wk = nc.dram_tensor("wk16", [128, CT, HPC * NOPE], BF16, kind="ExternalInput").ap()
    d_wv = nc.dram_tensor("wv16", [128, CT, HPC * VD], BF16, kind="ExternalInput").ap()
    d_wo = nc.dram_tensor("wo16", [128, HPC, HID], BF16, kind="ExternalInput").ap()
    d_cos = nc.dram_tensor("cosd", [128, SSH], BF16, kind="ExternalInput").ap()
    d_sin = nc.dram_tensor("sind", [128, SSH], BF16, kind="ExternalInput").ap()
    d_msk = nc.dram_tensor("maskadd", [128, 4, SB], F32, kind="ExternalInput").ap()
    d_rotq = nc.dram_tensor("rotq16", [128, 128], BF16, kind="ExternalInput").ap()
    d_out = nc.dram_tensor("out", [S, HID], BF16, kind="ExternalOutput").ap()

    with tile.TileContext(nc) as tc:
        _mla(tc, d_xt, d_wqa, d_wkva, d_wqb, d_wk, d_wv, d_wo, d_cos, d_sin,
             d_msk, d_rotq, d_out)
    nc.compile()
    return nc


def _mla(tc, d_xt, d_wqa, d_wkva, d_wqb, d_wk, d_wv, d_wo, d_cos, d_sin,
         d_msk, d_rotq, d_out):
    nc = tc.nc
    Exp = mybir.ActivationFunctionType.Exp
    Sqrt = mybir.ActivationFunctionType.Sqrt
    groups = [list(range(NC))]

    with nc.allow_low_precision(reason="bf16 pipeline"), \
         tc.tile_pool(name="pdram", bufs=1, space="DRAM") as pdram, \
         tc.tile_pool(name="pconst", bufs=1) as pc, \
         tc.tile_pool(name="pglob", bufs=1) as pg:
        # ---- DRAM bounce buffers for collectives ----
        ag_in = pdram.tile([KVL + RP, SSH], BF16)
        ag_out = pdram.tile([NC, KVL + RP, SSH], BF16)
        aa_in = [pdram.tile([NC, QD, SSH], BF16, name=f"aain{i}") for i in range(HPC)]
        aa_out = [pdram.tile([NC, QD, SSH], BF16, name=f"aaout{i}") for i in range(HPC)]

        # ---- small constants ----
        ones_c = pc.tile([128, 1], BF16)
        nc.vector.memset(ones_c, 1.0)
        ones_r = pc.tile([1, 128], BF16)
        nc.vector.memset(ones_r, 1.0)
        eps1 = pc.tile([1, 1], F32)
        nc.vector.memset(eps1, EPS)
        warm = pc.tile([1, 2], F32, name="actwarm")
        nc.scalar.activation(warm[0:1, 0:1], eps1[:], Sqrt)
        nc.scalar.activation(warm[0:1, 1:2], eps1[:], Exp)
        rotq = pc.tile([128, 128], BF16)
        nc.sync.dma_start(out=rotq, in_=d_rotq)
        cosd = pc.tile([128, SSH], BF16)
        nc.sync.dma_start(out=cosd, in_=d_cos)
        sind = pc.tile([128, SSH], BF16)
        nc.sync.dma_start(out=sind, in_=d_sin)

        # =============== stage A: shard projections ===============
        with tc.tile_pool(name="pw", bufs=1) as pw, \
             tc.tile_pool(name="pA", bufs=1) as pa, \
             tc.tile_pool(name="pAq", bufs=3) as paq, \
             tc.tile_pool(name="pAs", bufs=3) as pas, \
             tc.tile_pool(name="ppA", bufs=3, space="PSUM") as ppa, \
             tc.tile_pool(name="ppSt", bufs=2, space="PSUM") as ppst, \
             tc.tile_pool(name="ppM", bufs=2, space="PSUM") as ppm:
            # ---- stage-A weights: per-k-slice tiles so matmuls start early ----
            xt = pw.tile([128, KT, SSH], BF16)
            # Early (eager) loads on the SP HW queue: only what the first
            # ~20us of compute needs. Everything else is loaded via gpsimd
            # SWDGE triggers placed AFTER the AllGather in program order, so
            # those transfers enter the global DMA FIFO behind the
            # collective staging instead of ahead of it.
            wkva = pw.tile([128, KT, KVL + RP], BF16)
            for half in range(2):
                hk = slice(half * KT // 2, (half + 1) * KT // 2)
                nc.sync.dma_start(out=xt[:, hk, :], in_=d_xt[:, hk, :])
                nc.sync.dma_start(out=wkva[:, hk, :], in_=d_wkva[:, hk, :])
            wqa = pw.tile([128, KT, QL], BF16)
            # gate wqa-colA behind wkva via a write-after-read dep: the
            # reader consumes both the colA region and the wkva tail, so the
            # colA DMA (a writer of that region) must wait for wkva.
            gate = pc.tile([1, 2], BF16, name="gate")
            nc.vector.tensor_tensor(gate[0:1, 0:1], wqa[0:1, 0, 0:1],
                                    wkva[0:1, KT - 1, 0:1], mybir.AluOpType.mult)
            nc.scalar.dma_start(out=wqa[:, :, 0:QL // 2], in_=d_wqa[:, :, 0:QL // 2])
            wqb = pw.tile([128, QLT, H * QD], BF16)
            wk = pg.tile([128, CT, HPC * NOPE], BF16)
            wv = pg.tile([128, CT, HPC * VD], BF16)
            wo = pg.tile([128, HPC, HID], BF16)
            msk = pg.tile([128, 4, SB], F32)

            # --- kv LoRA-A ---
            ckvu = pa.tile([128, CT, SSH], BF16)
            kpe = pa.tile([RP, SSH], BF16)
            p_st = ppst.tile([1, SSH], F32, tag="st", name="cstat")
            sqc = pa.tile([128, CT, SSH], BF16, name="sqc")
            for m in range(CT + 1):
                mw = 128 if m < CT else RP
                p_a = ppa.tile([128, SSH], F32, tag="a")
                for k in range(KT):
                    nc.tensor.matmul(p_a[:mw, :], wkva[:, k, m * 128:m * 128 + mw],
                                     xt[:, k, :], start=(k == 0), stop=(k == KT - 1))
                if m < CT:
                    nc.vector.tensor_copy(ckvu[:, m, :], p_a[:])
                    nc.vector.tensor_mul(sqc[:, m, :], ckvu[:, m, :], ckvu[:, m, :])
                else:
                    nc.vector.tensor_copy(kpe[:], p_a[:mw, :])
            for m in range(CT):
                nc.tensor.matmul(p_st[:], ones_c[:], sqc[:, m, :],
                                 start=(m == 0), stop=(m == CT - 1))
            rms_c = pa.tile([1, SSH], BF16)
            nc.scalar.activation(rms_c[:], p_st[:], Sqrt, scale=1.0 / KVL,
                                 bias=eps1[:])
            p_bc = ppm.tile([128, SSH], F32, tag="m")
            nc.tensor.matmul(p_bc[:], ones_r[:], rms_c[:], start=True, stop=True)
            invc = pa.tile([128, SSH], BF16)
            nc.vector.reciprocal(invc[:], p_bc[:])
            ckv = pa.tile([128, CT, SSH], BF16)
            for m in range(CT):
                nc.vector.tensor_mul(ckv[:, m, :], ckvu[:, m, :], invc[:])
            nc.scalar.dma_start(
                out=ag_in[0:KVL, :].rearrange("(t p) c -> p t c", p=128),
                in_=ckv[:])
            # --- k_pe rope (scale folded: none needed in bf16) ---
            p_rk = ppm.tile([128, SSH], F32, tag="m", name="rotk")
            nc.tensor.matmul(p_rk[:RP, :], rotq[0:RP, 0:RP], kpe[:],
                             start=True, stop=True)
            rk16 = pas.tile([RP, SSH], BF16, tag="rk")
            nc.vector.tensor_copy(rk16[:], p_rk[:RP, :])
            t1 = pas.tile([RP, SSH], BF16, tag="t1")
            nc.vector.tensor_mul(t1[:], kpe[:], cosd[0:RP, :])
            t2 = pas.tile([RP, SSH], BF16, tag="t2")
            nc.vector.tensor_mul(t2[:], rk16[:], sind[0:RP, :])
            kpd = pa.tile([RP, SSH], BF16)
            nc.vector.tensor_add(kpd[:], t1[:], t2[:])
            nc.scalar.dma_start(out=ag_in[KVL:KVL + RP, :], in_=kpd[:])
            # --- collective #1: AllGather latent+kpe ---
            nc.gpsimd.collective_compute(
                "AllGather", mybir.AluOpType.bypass, replica_groups=groups,
                ins=[ag_in[:].opt()], outs=[ag_out[:].opt()])
            # deferred bulk weight loads, chained with write-after-read
            # gates so each transfer enters the exclusive DMA FIFO after the
            # AllGather staging and after the previous weight transfer.
            agmark = pc.tile([1, 2], BF16, name="agmark")
            nc.gpsimd.dma_start(out=agmark[0:1, 0:2],
                                in_=ag_in[KVL + RP - 1:KVL + RP, 0:2])
            Mul = mybir.AluOpType.mult

            def gate_read(region, token):
                g = pas.tile([1, 1], BF16, tag="g8")
                nc.vector.tensor_tensor(g[:], region, token, Mul)

            gate_read(wqa[0:1, 0, QL - 1:QL], ckv[0:1, CT - 1, 0:1])
            nc.gpsimd.dma_start(out=wqa[:, :, QL // 2:QL],
                                in_=d_wqa[:, :, QL // 2:QL])
            gate_read(wqb[0:1, 0, H * QD - 1:H * QD], agmark[0:1, 0:1])
            nc.gpsimd.dma_start(out=wqb[:, :, H * NOPE:H * QD],
                                in_=d_wqb[:, :, H * NOPE:H * QD])
            for q4 in range(4):
                qs = slice(q4 * H * NOPE // 4, (q4 + 1) * H * NOPE // 4)
                gate_read(wqb[0:1, 0, q4 * H * NOPE // 4:q4 * H * NOPE // 4 + 1],
                          wqb[0:1, 0, H * QD - 1:H * QD])
                nc.gpsimd.dma_start(out=wqb[:, :, qs], in_=d_wqb[:, :, qs])
            for wtile, dsrc in ((wk, d_wk), (wv, d_wv), (wo, d_wo), (msk, d_msk)):
                gate_read(wtile[0:1, 0, 0:1], wqb[0:1, 0, H * NOPE - 1:H * NOPE])
                nc.gpsimd.dma_start(out=wtile, in_=dsrc)

            # --- q LoRA-A ---
            qlu = pa.tile([128, QLT, SSH], BF16)
            p_qst = ppst.tile([1, SSH], F32, tag="st", name="qstat")
            sqq = pa.tile([128, QLT, SSH], BF16, name="sqq")
            for k in range(QLT):
                p_a = ppa.tile([128, SSH], F32, tag="a")
                for kk in range(KT):
                    nc.tensor.matmul(p_a[:], wqa[:, kk, k * 128:(k + 1) * 128],
                                     xt[:, kk, :], start=(kk == 0), stop=(kk == KT - 1))
                nc.vector.tensor_copy(qlu[:, k, :], p_a[:])
                nc.vector.tensor_mul(sqq[:, k, :], qlu[:, k, :], qlu[:, k, :])
            for k in range(QLT):
                nc.tensor.matmul(p_qst[:], ones_c[:], sqq[:, k, :],
                                 start=(k == 0), stop=(k == QLT - 1))
            rms_q = pa.tile([1, SSH], BF16)
            nc.scalar.activation(rms_q[:], p_qst[:], Sqrt, scale=1.0 / QL,
                                 bias=eps1[:])
            p_bq = ppm.tile([128, SSH], F32, tag="m")
            nc.tensor.matmul(p_bq[:], ones_r[:], rms_q[:], start=True, stop=True)
            invq = pa.tile([128, SSH], F32)
            nc.vector.reciprocal(invq[:], p_bq[:])

            # --- q_b for all heads: rope tiles (16..23) first so the rope
            # chain and the AllToAll staging DMAs start as early as possible;
            # nope tiles follow in parity order (A2A#1's inputs first).
            q16 = pa.tile([128, H + NC, SSH], BF16, name="q16")

            def qb_group(mt):
                p_q = ppa.tile([128, SSH], F32, tag="a")
                for k in range(QLT):
                    nc.tensor.matmul(p_q[:], wqb[:, k, mt * 128:(mt + 1) * 128],
                                     qlu[:, k, :], start=(k == 0), stop=(k == QLT - 1))
                nc.vector.tensor_mul(q16[:, mt, :], p_q[:], invq[:])

            for mt in range(H, H + NC):
                qb_group(mt)
            # rope rotate-half + cos/sin (inputs ready; no PE stalls)
            for d in range(NC):
                p_rq = ppm.tile([128, SSH], F32, tag="m")
                nc.tensor.matmul(p_rq[:], rotq[:], q16[:, H + d, :],
                                 start=True, stop=True)
                rq16 = pas.tile([128, SSH], BF16, tag="rk", name="rq16")
                nc.vector.tensor_copy(rq16[:], p_rq[:])
                t1q = pas.tile([128, SSH], BF16, tag="t1")
                nc.vector.tensor_mul(t1q[:], q16[:, H + d, :], cosd[:])
                t2q = pas.tile([128, SSH], BF16, tag="t2")
                nc.vector.tensor_mul(t2q[:], rq16[:], sind[:])
                nc.vector.tensor_add(q16[:, H + d, :], t1q[:], t2q[:])
            for mt in range(0, H, 2):
                qb_group(mt)
            nc.gpsimd.dma_start(
                out=aa_in[0][:, 0:NOPE, :].rearrange("j p c -> p j c"),
                in_=q16[:, 0:H:2, :].rearrange("p j c -> p j c"))
            nc.gpsimd.dma_start(
                out=aa_in[0][:, NOPE:QD, :].rearrange("j p c -> p j c"),
                in_=q16[0:RP, H:H + NC, :])
            nc.gpsimd.collective_compute(
                "AllToAll", mybir.AluOpType.bypass, replica_groups=groups,
                ins=[aa_in[0][:].opt()], outs=[aa_out[0][:].opt()])
            for mt in range(1, H, 2):
                qb_group(mt)
            nc.gpsimd.dma_start(
                out=aa_in[1][:, 0:NOPE, :].rearrange("j p c -> p j c"),
                in_=q16[:, 1:H:2, :].rearrange("p j c -> p j c"))
            nc.gpsimd.dma_start(
                out=aa_in[1][:, NOPE:QD, :].rearrange("j p c -> p j c"),
                in_=q16[RP:128, H:H + NC, :])
            nc.gpsimd.collective_compute(
                "AllToAll", mybir.AluOpType.bypass, replica_groups=groups,
                ins=[aa_in[1][:].opt()], outs=[aa_out[1][:].opt()])

        # =============== stage B: head-local attention ===============
        with tc.tile_pool(name="pB", bufs=1) as pb, \
             tc.tile_pool(name="pBe", bufs=13) as pbe, \
             tc.tile_pool(name="pBo", bufs=4) as pbo, \
             tc.tile_pool(name="pBn", bufs=4) as pbn, \
             tc.tile_pool(name="ppS", bufs=3, space="PSUM") as pps, \
             tc.tile_pool(name="ppO", bufs=2, space="PSUM") as ppo, \
             tc.tile_pool(name="ppD", bufs=1, space="PSUM") as ppd, \
             tc.tile_pool(name="ppC", bufs=2, space="PSUM") as ppc:
            ckvg = pb.tile([128, CT, S], BF16)
            for t in range(CT):
                nc.gpsimd.dma_start(
                    out=ckvg[:, t, :].rearrange("p (j c) -> p j c", j=NC),
                    in_=ag_out[:, t * 128:(t + 1) * 128, :].rearrange(
                        "j p c -> p j c"))
            kpdg = pb.tile([RP, S], BF16)
            nc.gpsimd.dma_start(out=kpdg[:].rearrange("p (j c) -> p j c", j=NC),
                              in_=ag_out[:, KVL:KVL + RP, :].rearrange(
                                  "j p c -> p j c"))
            qt = [pb.tile([128, S], BF16, name=f"qt{h}") for h in range(HPC)]
            qpt = [pb.tile([RP, S], BF16, name=f"qpt{h}") for h in range(HPC)]

            def unpack_q(h):
                nc.gpsimd.dma_start(
                    out=qt[h][:].rearrange("p (j c) -> p j c", j=NC),
                    in_=aa_out[h][:, 0:NOPE, :].rearrange("j p c -> p j c"))
                nc.gpsimd.dma_start(
                    out=qpt[h][:].rearrange("p (j c) -> p j c", j=NC),
                    in_=aa_out[h][:, NOPE:QD, :].rearrange("j p c -> p j c"))

            # --- kv_b: kn per head, v (both heads) keys-on-partitions ---
            kn = [pb.tile([128, S], BF16, name=f"kn{h}") for h in range(HPC)]
            for h in range(HPC):
                for cb in range(S // SSH):
                    p_k = ppc.tile([128, SSH], F32, tag="c")
                    for t in range(CT):
                        nc.tensor.matmul(p_k[:], wk[:, t, h * NOPE:(h + 1) * NOPE],
                                         ckvg[:, t, cb * SSH:(cb + 1) * SSH],
                                         start=(t == 0), stop=(t == CT - 1))
                    nc.any.tensor_copy(kn[h][:, cb * SSH:(cb + 1) * SSH], p_k[:])
            vst = pb.tile([128, S // 128, HPC * VD], BF16)
            for sb in range(S // 128):
                p_v = ppc.tile([128, HPC * VD], F32, tag="c")
                for t in range(CT):
                    nc.tensor.matmul(p_v[:], ckvg[:, t, sb * 128:(sb + 1) * 128],
                                     wv[:, t, :], start=(t == 0), stop=(t == CT - 1))
                nc.any.tensor_copy(vst[:, sb, :], p_v[:])

            # --- attention: heads outer (matches AllToAll arrival).
            # Software-pipelined: AV/den for ik are issued after the scores
            # of ik+1 so the PE never stalls on the exp; the per-(qb,h)
            # normalization finisher is deferred into the next iteration's
            # matmul stream.
            ao = pb.tile([128, NSB, HPC, SB], BF16)
            pending = None

            def finisher(fin):
                h, qb, p_o, p_d = fin
                den = pbn.tile([1, SB], BF16, tag="den")
                nc.vector.tensor_copy(den[:], p_d[:])
                p_b = ppc.tile([128, SB], F32, tag="c", name="bcast")
                nc.tensor.matmul(p_b[:], ones_r[:], den[:], start=True, stop=True)
                rec = pbn.tile([128, SB], F32, tag="rec")
                nc.vector.reciprocal(rec[:], p_b[:])
                nc.vector.tensor_mul(ao[:, qb, h, :], p_o[:], rec[:])

            def oproj(qb):
                for st in range(SB // 128):
                    sc = slice(qb * SB + st * 128, qb * SB + (st + 1) * 128)
                    ot = pbo.tile([128, HID], BF16, tag="ot")
                    for nb in range(HID // SB):
                        ncols = bass.ts(nb, SB)
                        p_c = ppc.tile([128, SB], F32, tag="c")
                        for hh in range(HPC):
                            nc.tensor.matmul(
                                p_c[:], ao[:, qb, hh, st * 128:(st + 1) * 128],
                                wo[:, hh, ncols],
                                start=(hh == 0), stop=(hh == HPC - 1))
                        nc.vector.tensor_copy(ot[:, ncols], p_c[:])
                    nc.sync.dma_start(out=d_out[sc, :], in_=ot[:])

            for h in range(HPC):
                unpack_q(h)
                for qb in range(NSB):
                    qcols = bass.ts(qb, SB)
                    nk = 4 * (qb + 1)
                    p_o = ppo.tile([128, SB], F32, tag="o")
                    p_d = ppd.tile([1, SB], F32, tag="d")
                    ework = []

                    def av_den(pik, pe_):
                        nc.tensor.matmul(p_o[:], vst[:, pik, h * VD:(h + 1) * VD],
                                         pe_[:], start=(pik == 0),
                                         stop=(pik == nk - 1))
                        nc.tensor.matmul(p_d[:], ones_c[:], pe_[:],
                                         start=(pik == 0), stop=(pik == nk - 1))

                    for ik in range(nk):
                        kc = slice(ik * 128, (ik + 1) * 128)
                        p_s = pps.tile([128, SB], F32, tag="s")
                        nc.tensor.matmul(p_s[:], kn[h][:, kc], qt[h][:, qcols],
                                         start=True, stop=False)
                        nc.tensor.matmul(p_s[:], kpdg[:, kc], qpt[h][:, qcols],
                                         start=False, stop=True)
                        if ik == 3 and pending is not None:
                            fin, oqb = pending
                            finisher(fin)
                            pending = None
                            if oqb is not None:
                                oproj(oqb)
                        if len(ework) == 5:
                            av_den(*ework.pop(0))
                        r = ik - 4 * qb
                        if r >= 0:
                            nc.vector.tensor_add(p_s[:], p_s[:], msk[:, r, :])
                        e = pbe.tile([128, SB], BF16, tag="e")
                        nc.scalar.activation(e[:], p_s[:], Exp, scale=SCALE)
                        ework.append((ik, e))
                    for item in ework:
                        av_den(*item)
                    pending = ((h, qb, p_o, p_d),
                               qb if h == HPC - 1 else None)
            fin, oqb = pending
            finisher(fin)
            if oqb is not None:
                oproj(oqb)


def _host_constants():
    inv_freq = 1.0 / (ROPE_THETA ** (np.arange(0, RP, 2, dtype=np.float32) / RP))
    t = np.arange(S, dtype=np.float32)
    freqs = np.outer(t, inv_freq)
    emb = np.concatenate([freqs, freqs], -1)          # [S, 64]
    cos, sin = np.cos(emb), np.sin(emb)
    cosd = np.concatenate([cos.T, cos.T], 0).astype(np.float32)   # [128, S]
    sind = np.concatenate([sin.T, sin.T], 0).astype(np.float32)

    # additive causal mask for diagonal 128-key blocks: [128, 4, 512]
    mska = np.zeros((128, 4, SB), np.float32)
    for r in range(4):
        for p in range(128):
            mska[p, r, :p + 128 * r] = NEG
    # rotate-half as matmul lhsT: same as baseline
    Q = np.zeros((RP, RP), np.float32)
    for i in range(RP // 2):
        Q[i, i + RP // 2] = -1.0
        Q[i + RP // 2, i] = 1.0
    P = np.zeros((128, 128), np.float32)
    P[:RP, :RP] = Q
    P[RP:, RP:] = Q
    rotq = P.T.copy()
    return cosd, sind, mska, rotq


def _tile3(w, kt):
    """[kt*128, F] -> [128, kt, F]"""
    return np.ascontiguousarray(
        w.reshape(kt, 128, w.shape[1]).transpose(1, 0, 2))


def kernel(hidden_states, w_q_a, q_a_weight, w_q_b, w_kv_a, kv_a_weight,
           w_kv_b, w_o):
    global LAST_RESULT
    if "nc" not in _CACHE:
        _CACHE["nc"] = _build_program()
    nc = _CACHE["nc"]

    x = np.asarray(hidden_states, np.float32)[0]       # [S, 2048]
    xt = np.ascontiguousarray(x.T)                     # [2048, S]
    wqa_t = np.asarray(w_q_a, np.float32).T            # [HID, QL]
    wkva_t = np.asarray(w_kv_a, np.float32).T          # [HID, KVL+RP]
    wqb_eff = np.asarray(w_q_b, np.float32) * np.asarray(q_a_weight, np.float32)[None, :]
    wkvb_eff = np.asarray(w_kv_b, np.float32) * np.asarray(kv_a_weight, np.float32)[None, :]
    won = np.asarray(w_o, np.float32)                  # [HID, H*VD]

    # q_b output feature permutation: nope head-major, then rope packed 2/tile
    perm = np.zeros(H * QD, np.int64)
    for h in range(H):
        perm[h * NOPE:(h + 1) * NOPE] = h * QD + np.arange(NOPE)
    base = H * NOPE
    for d in range(NC):
        for j in range(HPC):
            hh = 2 * d + j
            perm[base + d * 128 + j * RP: base + d * 128 + (j + 1) * RP] = \
                hh * QD + NOPE + np.arange(RP)
    wqb_p = wqb_eff[perm, :]                           # [3072, QL]

    cosd, sind, mska, rotq = _host_constants()

    wqa16 = _tile3(wqa_t, KT).astype(NPBF)
    wkva16 = _tile3(wkva_t, KT).astype(NPBF)
    wqb16 = _tile3(np.ascontiguousarray(wqb_p.T), QLT).astype(NPBF)
    rotq16 = rotq.astype(NPBF)

    shared = {"wqa16": wqa16, "wkva16": wkva16, "wqb16": wqb16,
              "maskadd": mska, "rotq16": rotq16}

    in_maps = []
    for c in range(NC):
        h0, h1 = HPC * c, HPC * c + 1
        wk_t = np.concatenate(
            [wkvb_eff[h * (NOPE + VD):h * (NOPE + VD) + NOPE] for h in (h0, h1)],
            0).T                                        # [KVL, 256]
        wv_t = np.concatenate(
            [wkvb_eff[h * (NOPE + VD) + NOPE:(h + 1) * (NOPE + VD)] for h in (h0, h1)],
            0).T                                        # [KVL, 256]
        wo_t = np.stack(
            [np.ascontiguousarray(won[:, h * VD:(h + 1) * VD].T) for h in (h0, h1)],
            1)                                          # [128, 2, HID]
        cols = slice(c * SSH, (c + 1) * SSH)
        im = dict(shared)
        im.update({
            "xt16": _tile3(np.ascontiguousarray(xt[:, cols]), KT).astype(NPBF),
            "wk16": _tile3(wk_t, CT).astype(NPBF),
            "wv16": _tile3(wv_t, CT).astype(NPBF),
            "wo16": np.ascontiguousarray(wo_t).astype(NPBF),
            "cosd": np.ascontiguousarray(cosd[:, cols]).astype(NPBF),
            "sind": np.ascontiguousarray(sind[:, cols]).astype(NPBF),
        })
        in_maps.append(im)

    res = run_bass_kernel_spmd(nc, in_maps, list(range(NC)))
    LAST_RESULT = res
    out = np.zeros((S, HID), np.float32)
    for c in range(NC):
        out += np.asarray(res.results[c]["out"]).astype(np.float32)
    return out.reshape(1, S, HID)
